# revision 4
# baseline (speedup 1.0000x reference)
"""Trainium2 Bass kernel for batched 16-head attention (B=8, N=1024, D=1024).

Sharding: data-parallel over batch — one batch element per NeuronCore (8 cores).

Per-core pipeline (all matmuls in float32r, fp32 storage):
  1. q,k projected feature-major (qT/kT = W.T @ x.T), v projected seq-major,
     with a ones-column appended per head so the attention-weight row-sums
     (softmax denominators) fall out of the same matmul as out = e.T @ v_aug.
  2. scores computed transposed (scores_T[j, i] = k . q) so the mask penalty
     is a per-partition bias fused into the ScalarE exp together with the
     1/sqrt(d) scale: e = exp(0.125 * scores_T + pen[j]).
  3. masked key rows are dropped entirely (host gathers kept rows; a masked
     row's exp(-10000 + s) is exactly 0.0 in f32, so dropping it is exact).
  4. out_raw_T accumulated per head, PE-transposed back to natural layout,
     normalized by the row-sums, DMA'd out.
"""

import sys

sys.path.insert(0, "/opt/trn_rl_repo")

import numpy as np

import concourse.bass as bass
import concourse.bacc as bacc
import concourse.mybir as mybir
from concourse.tile import TileContext
from concourse.bass_utils import run_bass_kernel_spmd

B = 8
N = 1024          # sequence length (queries)
D = 1024          # model dim
H = 16            # heads
DH = 64           # head dim
NPAIR = H // 2    # head pairs (2 heads share one 128-row feature tile)
P = 128
F32 = mybir.dt.float32
F32R = mybir.dt.float32r
EXP = mybir.ActivationFunctionType.Exp

_CACHE = {}


def build_nc(n_j, repeat=0):
    """Build the per-core Bass graph.

    n_j: padded count of kept key rows (multiple of 128). If n_j == N the
         k/v projections read the full xT input (no separate gathered input).
    repeat: if > 0, wrap the whole compute in a For_i timing loop.
    """
    n_jc = n_j // 128
    share_xt = n_j == N

    nc = bacc.Bacc(None, target_bir_lowering=False)
    xt_ext = nc.declare_dram_parameter("xt", [D, N], F32, isOutput=False)
    if not share_xt:
        xtkv_ext = nc.declare_dram_parameter("xtkv", [D, n_j], F32, isOutput=False)
    w_ext = nc.declare_dram_parameter("w", [D, 3 * D], F32, isOutput=False)
    pen_ext = nc.declare_dram_parameter("pen", [P, n_jc], F32, isOutput=False)
    id_ext = nc.declare_dram_parameter("ident", [P, P], F32, isOutput=False)
    out_ext = nc.declare_dram_parameter("out", [N, D], F32, isOutput=True)

    with TileContext(nc) as tc:
        with (
            tc.tile_pool(name="const", bufs=1) as const_pool,
            tc.tile_pool(name="xt", bufs=1) as xt_pool,
            tc.tile_pool(name="qk", bufs=1) as qk_pool,
            tc.tile_pool(name="vnat", bufs=1) as v_pool,
            tc.tile_pool(name="wq", bufs=3) as w_pool,
            tc.tile_pool(name="wv", bufs=1) as wv_pool,
            tc.tile_pool(name="e", bufs=3) as e_pool,
            tc.tile_pool(name="ot", bufs=2) as ot_pool,
            tc.tile_pool(name="ob", bufs=3) as ob_pool,
            tc.tile_pool(name="psb", bufs=2, space="PSUM") as psb_pool,
            tc.tile_pool(name="pso", bufs=2, space="PSUM") as pso_pool,
        ):
            pen_sb = const_pool.tile([P, n_jc], F32, tag="pen")
            nc.sync.dma_start(out=pen_sb[:], in_=pen_ext[:])
            id_sb = const_pool.tile([P, P], F32, tag="ident")
            nc.sync.dma_start(out=id_sb[:], in_=id_ext[:])

            # ---- load xT (d-major x) ----
            xt_sb = []
            for dc in range(8):
                t = xt_pool.tile([P, N], F32R, tag=f"xt{dc}")
                nc.sync.dma_start(
                    out=t[:], in_=xt_ext[dc * P:(dc + 1) * P, :].bitcast(F32R)
                )
                xt_sb.append(t)
            if share_xt:
                xtkv_sb = xt_sb
            else:
                xtkv_sb = []
                for dc in range(8):
                    t = xt_pool.tile([P, n_j], F32R, tag=f"xtkv{dc}")
                    nc.sync.dma_start(
                        out=t[:], in_=xtkv_ext[dc * P:(dc + 1) * P, :].bitcast(F32R)
                    )
                    xtkv_sb.append(t)

            def body():
                # ---- q/k projection, feature-major: qkT[f, n] = W[:, f].T @ xT ----
                # fc 0..7 -> q pairs (length N), fc 8..15 -> k pairs (length n_j)
                qk_sb = []
                for fc in range(16):
                    n_cols = N if fc < 8 else n_j
                    src = xt_sb if fc < 8 else xtkv_sb
                    w_sb = w_pool.tile([P, D], F32R, tag="w")
                    for dc in range(8):
                        nc.sync.dma_start(
                            out=w_sb[:, dc * P:(dc + 1) * P],
                            in_=w_ext[dc * P:(dc + 1) * P,
                                      fc * P:(fc + 1) * P].bitcast(F32R),
                        )
                    ps = psb_pool.tile([P, N], F32, tag="big")
                    for c0 in range(0, n_cols, 512):
                        c1 = min(c0 + 512, n_cols)
                        for dc in range(8):
                            nc.tensor.matmul(
                                ps[:, c0:c1],
                                w_sb[:, dc * P:(dc + 1) * P],
                                src[dc][:, c0:c1],
                                start=(dc == 0),
                                stop=(dc == 7),
                            )
                    dst = qk_pool.tile([P, n_cols], F32R, tag=f"qk{fc}")
                    nc.vector.tensor_copy(dst[:], ps[:, :n_cols])
                    qk_sb.append(dst)

                # ---- v projection, seq-major with ones column per head ----
                # v_nat[jc] layout: [128 j, 16*65] = per head 64 v-cols + 1 ones-col
                v_nat = []
                for jc in range(n_jc):
                    t = v_pool.tile([P, H * 65], F32R, tag=f"v{jc}")
                    nc.vector.memset(
                        t.rearrange("p (h c) -> p h c", c=65)[:, :, 64:65].bitcast(F32),
                        1.0,
                    )
                    v_nat.append(t)
                for hv in range(2):
                    wv_sb = []
                    for dc in range(8):
                        t = wv_pool.tile([P, 512], F32R, tag=f"wv{dc}")
                        nc.sync.dma_start(
                            out=t[:],
                            in_=w_ext[dc * P:(dc + 1) * P,
                                      2048 + hv * 512:2048 + (hv + 1) * 512
                                      ].bitcast(F32R),
                        )
                        wv_sb.append(t)
                    for jc in range(n_jc):
                        ps = psb_pool.tile([P, 512], F32, tag="big")
                        for dc in range(8):
                            nc.tensor.matmul(
                                ps[:],
                                xtkv_sb[dc][:, jc * P:(jc + 1) * P],
                                wv_sb[dc][:],
                                start=(dc == 0),
                                stop=(dc == 7),
                            )
                        dstv = v_nat[jc].rearrange("p (h c) -> p h c", c=65)
                        nc.vector.tensor_copy(
                            dstv[:, hv * 8:(hv + 1) * 8, 0:64],
                            ps[:].rearrange("p (h c) -> p h c", c=64),
                        )

                # ---- attention per head pair ----
                for p in range(NPAIR):
                    qT = qk_sb[p]
                    kT = qk_sb[8 + p]
                    ha, hb = 2 * p, 2 * p + 1
                    for ih in range(2):
                        i0 = ih * 512
                        ps_s = psb_pool.tile([P, 1024], F32, tag="big")
                        ps_o = pso_pool.tile([65, 1024], F32, tag="ps_o")
                        for jc in range(n_jc):
                            nc.tensor.matmul(
                                ps_s[:, 0:512],
                                kT[0:64, jc * P:(jc + 1) * P],
                                qT[0:64, i0:i0 + 512],
                                start=True, stop=True,
                            )
                            nc.tensor.matmul(
                                ps_s[:, 512:1024],
                                kT[64:128, jc * P:(jc + 1) * P],
                                qT[64:128, i0:i0 + 512],
                                start=True, stop=True,
                            )
                            e_sb = e_pool.tile([P, 1024], F32R, tag="e")
                            nc.scalar.activation(
                                e_sb[:], ps_s[:], EXP,
                                bias=pen_sb[:, jc:jc + 1], scale=0.125,
                            )
                            nc.tensor.matmul(
                                ps_o[:, 0:512],
                                v_nat[jc][:, ha * 65:(ha + 1) * 65],
                                e_sb[:, 0:512],
                                start=(jc == 0), stop=(jc == n_jc - 1),
                            )
                            nc.tensor.matmul(
                                ps_o[:, 512:1024],
                                v_nat[jc][:, hb * 65:(hb + 1) * 65],
                                e_sb[:, 512:1024],
                                start=(jc == 0), stop=(jc == n_jc - 1),
                            )
                        ot = ot_pool.tile([65, 1024], F32, tag="ot")
                        nc.vector.tensor_copy(ot[:], ps_o[:])
                        ps_t = psb_pool.tile([P, 1024], F32, tag="big")
                        for ic in range(4):
                            c = ic * P
                            t0 = ic * 256
                            nc.tensor.transpose(
                                ps_t[:, t0:t0 + 65],
                                ot[:, c:c + P], id_sb[0:65, 0:65]
                            )
                            nc.tensor.transpose(
                                ps_t[:, t0 + 128:t0 + 193],
                                ot[:, 512 + c:512 + c + P], id_sb[0:65, 0:65]
                            )
                        for ic in range(4):
                            t0 = ic * 256
                            ob = ob_pool.tile([P, P], F32, tag="ob")
                            rc = ob_pool.tile([P, 2], F32, tag="rc")
                            nc.vector.reciprocal(rc[:, 0:1], ps_t[:, t0 + 64:t0 + 65])
                            nc.vector.reciprocal(rc[:, 1:2], ps_t[:, t0 + 192:t0 + 193])
                            nc.vector.tensor_scalar_mul(
                                ob[:, 0:64], ps_t[:, t0:t0 + 64], rc[:, 0:1]
                            )
                            nc.vector.tensor_scalar_mul(
                                ob[:, 64:128], ps_t[:, t0 + 128:t0 + 192], rc[:, 1:2]
                            )
                            nc.sync.dma_start(
                                out=out_ext[i0 + ic * P:i0 + (ic + 1) * P,
                                            p * P:(p + 1) * P],
                                in_=ob[:],
                            )

            if repeat > 0:
                with tc.For_i(0, repeat, 1):
                    body()
            else:
                body()

    nc.compile()
    return nc


def _host_prep(x, mask, w_qkv):
    """Shard + lay out inputs per core. Returns (in_maps, n_j)."""
    x = np.ascontiguousarray(x, dtype=np.float32)
    mask = np.asarray(mask)
    w_qkv = np.ascontiguousarray(w_qkv, dtype=np.float32)

    # kept key rows per batch: j=0 always kept, then mask over rows 1..N-1
    keep = np.concatenate([np.ones((B, 1), dtype=bool), mask.astype(bool)], axis=1)
    counts = keep.sum(axis=1)
    n_j = int(np.ceil(counts.max() / 128.0) * 128)
    n_j = min(n_j, N)

    ident = np.eye(P, dtype=np.float32)
    in_maps = []
    for b in range(B):
        xt = np.ascontiguousarray(x[b].T)               # [D, N]
        idx = np.nonzero(keep[b])[0]
        m = {"xt": xt, "w": w_qkv, "ident": ident}
        pen = np.full(n_j, -10000.0, dtype=np.float32)  # padding rows masked out
        pen[: len(idx)] = 0.0
        m["pen"] = np.ascontiguousarray(pen.reshape(n_j // 128, 128).T)  # [128, n_jc]
        if n_j == N:
            # no gather: full rows, penalty by original position
            penf = np.full(N, -10000.0, dtype=np.float32)
            penf[keep[b]] = 0.0
            m["pen"] = np.ascontiguousarray(penf.reshape(N // 128, 128).T)
        else:
            xkv = np.zeros((D, n_j), dtype=np.float32)
            xkv[:, : len(idx)] = xt[:, idx]
            m["xtkv"] = xkv
        in_maps.append(m)
    return in_maps, n_j


def kernel(x, mask, w_qkv):
    in_maps, n_j = _host_prep(x, mask, w_qkv)
    if n_j not in _CACHE:
        _CACHE[n_j] = build_nc(n_j)
    nc = _CACHE[n_j]
    res = run_bass_kernel_spmd(nc, in_maps, core_ids=list(range(B)))
    out = np.stack([np.asarray(res.results[i]["out"]) for i in range(B)], axis=0)
    return out.astype(np.float32)


if __name__ == "__main__":
    rng = np.random.default_rng(0)
    x = rng.standard_normal((B, N, D), dtype=np.float32)
    mask = rng.integers(0, 2, size=(B, N - 1)).astype(np.int32)
    w = (rng.standard_normal((D, 3 * D), dtype=np.float32) * D ** -0.5).astype(np.float32)
    out = kernel(x=x, mask=mask, w_qkv=w)
    print("out", out.shape, out.dtype, float(np.abs(out).mean()))


# revision 6
# speedup vs baseline: 1.4348x; 1.4348x over previous
"""Trainium2 Bass kernel for batched 16-head attention (B=8, N=1024, D=1024).

Sharding: data-parallel over batch — one batch element per NeuronCore (8 cores).

Per-core pipeline (all matmuls in float32r, fp32 storage):
  1. q,k projected feature-major (qT/kT = W.T @ x.T), v projected seq-major,
     with a ones-column appended per head so the attention-weight row-sums
     (softmax denominators) fall out of the same matmul as out = e.T @ v_aug.
  2. scores computed transposed (scores_T[j, i] = k . q) so the mask penalty
     is a per-partition bias fused into the ScalarE exp together with the
     1/sqrt(d) scale: e = exp(0.125 * scores_T + pen[j]).
  3. masked key rows are dropped entirely (host gathers kept rows; a masked
     row's exp(-10000 + s) is exactly 0.0 in f32, so dropping it is exact).
  4. out_raw_T accumulated per head, PE-transposed back to natural layout,
     normalized by the row-sums, DMA'd out.
"""

import sys

sys.path.insert(0, "/opt/trn_rl_repo")

import numpy as np

import concourse.bass as bass
import concourse.bacc as bacc
import concourse.mybir as mybir
from concourse.tile import TileContext
from concourse.bass_utils import run_bass_kernel_spmd

B = 8
N = 1024          # sequence length (queries)
D = 1024          # model dim
H = 16            # heads
DH = 64           # head dim
NPAIR = H // 2    # head pairs (2 heads share one 128-row feature tile)
P = 128
F32 = mybir.dt.float32
F32R = mybir.dt.float32r
EXP = mybir.ActivationFunctionType.Exp

_CACHE = {}


def build_nc(n_j, repeat=0):
    """Build the per-core Bass graph.

    n_j: padded count of kept key rows (multiple of 128). If n_j == N the
         k/v projections read the full xT input (no separate gathered input).
    repeat: if > 0, wrap the whole compute in a For_i timing loop.
    """
    n_jc = n_j // 128
    share_xt = n_j == N

    nc = bacc.Bacc(None, target_bir_lowering=False)
    xt_ext = nc.declare_dram_parameter("xt", [D, N], F32, isOutput=False)
    if not share_xt:
        xtkv_ext = nc.declare_dram_parameter("xtkv", [D, n_j], F32, isOutput=False)
    w_ext = nc.declare_dram_parameter("w", [D, 3 * D], F32, isOutput=False)
    pen_ext = nc.declare_dram_parameter("pen", [P, n_jc], F32, isOutput=False)
    id_ext = nc.declare_dram_parameter("ident", [P, P], F32, isOutput=False)
    out_ext = nc.declare_dram_parameter("out", [N, D], F32, isOutput=True)

    with TileContext(nc) as tc:
        with (
            tc.tile_pool(name="const", bufs=1) as const_pool,
            tc.tile_pool(name="xt", bufs=1) as xt_pool,
            tc.tile_pool(name="qk", bufs=1) as qk_pool,
            tc.tile_pool(name="vnat", bufs=1) as v_pool,
            tc.tile_pool(name="wq", bufs=3) as w_pool,
            tc.tile_pool(name="wv", bufs=1) as wv_pool,
            tc.tile_pool(name="e", bufs=4) as e_pool,
            tc.tile_pool(name="ot", bufs=4) as ot_pool,
            tc.tile_pool(name="ob", bufs=3) as ob_pool,
            tc.tile_pool(name="psb", bufs=2, space="PSUM") as psb_pool,
            tc.tile_pool(name="pso", bufs=2, space="PSUM") as pso_pool,
        ):
            pen_sb = const_pool.tile([P, n_jc], F32, tag="pen")
            nc.sync.dma_start(out=pen_sb[:], in_=pen_ext[:])
            id_sb = const_pool.tile([P, P], F32, tag="ident")
            nc.sync.dma_start(out=id_sb[:], in_=id_ext[:])

            # ---- load xT (d-major x) ----
            xt_sb = []
            for dc in range(8):
                t = xt_pool.tile([P, N], F32R, tag=f"xt{dc}")
                for h0 in range(0, N, 512):
                    nc.sync.dma_start(
                        out=t[:, h0:h0 + 512],
                        in_=xt_ext[dc * P:(dc + 1) * P, h0:h0 + 512].bitcast(F32R),
                    )
                xt_sb.append(t)
            if share_xt:
                xtkv_sb = xt_sb
            else:
                xtkv_sb = []
                for dc in range(8):
                    t = xt_pool.tile([P, n_j], F32R, tag=f"xtkv{dc}")
                    nc.sync.dma_start(
                        out=t[:], in_=xtkv_ext[dc * P:(dc + 1) * P, :].bitcast(F32R)
                    )
                    xtkv_sb.append(t)

            def body():
                # ---- q/k projection, feature-major: qkT[f, n] = W[:, f].T @ xT ----
                # fc 0..7 -> q pairs (length N), fc 8..15 -> k pairs (length n_j)
                qk_sb = []
                for fc in range(16):
                    n_cols = N if fc < 8 else n_j
                    src = xt_sb if fc < 8 else xtkv_sb
                    w_sb = w_pool.tile([P, D], F32R, tag="w")
                    for dc in range(8):
                        nc.sync.dma_start(
                            out=w_sb[:, dc * P:(dc + 1) * P],
                            in_=w_ext[dc * P:(dc + 1) * P,
                                      fc * P:(fc + 1) * P].bitcast(F32R),
                        )
                    ps = psb_pool.tile([P, N], F32, tag="big")
                    for c0 in range(0, n_cols, 512):
                        c1 = min(c0 + 512, n_cols)
                        for dc in range(8):
                            nc.tensor.matmul(
                                ps[:, c0:c1],
                                w_sb[:, dc * P:(dc + 1) * P],
                                src[dc][:, c0:c1],
                                start=(dc == 0),
                                stop=(dc == 7),
                            )
                    dst = qk_pool.tile([P, n_cols], F32R, tag=f"qk{fc}")
                    nc.vector.tensor_copy(dst[:], ps[:, :n_cols])
                    qk_sb.append(dst)

                # ---- v projection, seq-major with ones column per head ----
                # v_nat[jc] layout: [128 j, 16*65] = per head 64 v-cols + 1 ones-col
                v_nat = []
                for jc in range(n_jc):
                    t = v_pool.tile([P, H * 65], F32R, tag=f"v{jc}")
                    nc.vector.memset(
                        t.rearrange("p (h c) -> p h c", c=65)[:, :, 64:65].bitcast(F32),
                        1.0,
                    )
                    v_nat.append(t)
                for hv in range(2):
                    wv_sb = []
                    for dc in range(8):
                        t = wv_pool.tile([P, 512], F32R, tag=f"wv{dc}")
                        nc.sync.dma_start(
                            out=t[:],
                            in_=w_ext[dc * P:(dc + 1) * P,
                                      2048 + hv * 512:2048 + (hv + 1) * 512
                                      ].bitcast(F32R),
                        )
                        wv_sb.append(t)
                    for jc in range(n_jc):
                        ps = psb_pool.tile([P, 512], F32, tag="big")
                        for dc in range(8):
                            nc.tensor.matmul(
                                ps[:],
                                xtkv_sb[dc][:, jc * P:(jc + 1) * P],
                                wv_sb[dc][:],
                                start=(dc == 0),
                                stop=(dc == 7),
                            )
                        dstv = v_nat[jc].rearrange("p (h c) -> p h c", c=65)
                        nc.vector.tensor_copy(
                            dstv[:, hv * 8:(hv + 1) * 8, 0:64],
                            ps[:].rearrange("p (h c) -> p h c", c=64),
                        )

                # ---- attention per head pair ----
                # The two i-half chains (ih=0,1) are interleaved so one
                # chain's PE work hides the other's exp latency; the
                # transpose/normalize epilogue runs one block behind so it
                # slots into PE gaps of the next block.
                def epilogue(p, ih, ot):
                    i0 = ih * 512
                    ps_t = psb_pool.tile([P, 1024], F32, tag="big")
                    for ic in range(4):
                        c = ic * P
                        t0 = ic * 256
                        nc.tensor.transpose(
                            ps_t[:, t0:t0 + 65],
                            ot[:, c:c + P], id_sb[0:65, 0:65]
                        )
                        nc.tensor.transpose(
                            ps_t[:, t0 + 128:t0 + 193],
                            ot[:, 512 + c:512 + c + P], id_sb[0:65, 0:65]
                        )
                    for ic in range(4):
                        t0 = ic * 256
                        ob = ob_pool.tile([P, P], F32, tag="ob")
                        rc = ob_pool.tile([P, 2], F32, tag="rc")
                        nc.vector.reciprocal(rc[:, 0:1], ps_t[:, t0 + 64:t0 + 65])
                        nc.vector.reciprocal(rc[:, 1:2], ps_t[:, t0 + 192:t0 + 193])
                        nc.vector.tensor_scalar_mul(
                            ob[:, 0:64], ps_t[:, t0:t0 + 64], rc[:, 0:1]
                        )
                        nc.vector.tensor_scalar_mul(
                            ob[:, 64:128], ps_t[:, t0 + 128:t0 + 192], rc[:, 1:2]
                        )
                        nc.sync.dma_start(
                            out=out_ext[i0 + ic * P:i0 + (ic + 1) * P,
                                        p * P:(p + 1) * P],
                            in_=ob[:],
                        )

                pending = None
                for p in range(NPAIR):
                    qT = qk_sb[p]
                    kT = qk_sb[8 + p]
                    ha, hb = 2 * p, 2 * p + 1
                    ps_s = [psb_pool.tile([P, 1024], F32, tag="big", name=f"ps_s{ih}")
                            for ih in range(2)]
                    ps_o = [pso_pool.tile([65, 1024], F32, tag="ps_o", name=f"ps_o{ih}")
                            for ih in range(2)]
                    for jc in range(n_jc):
                        e_sb = [None, None]
                        for ih in range(2):
                            i0 = ih * 512
                            nc.tensor.matmul(
                                ps_s[ih][:, 0:512],
                                kT[0:64, jc * P:(jc + 1) * P],
                                qT[0:64, i0:i0 + 512],
                                start=True, stop=True,
                            )
                            nc.tensor.matmul(
                                ps_s[ih][:, 512:1024],
                                kT[64:128, jc * P:(jc + 1) * P],
                                qT[64:128, i0:i0 + 512],
                                start=True, stop=True,
                            )
                            e_sb[ih] = e_pool.tile([P, 1024], F32R, tag="e", name=f"e{ih}")
                            nc.scalar.activation(
                                e_sb[ih][:], ps_s[ih][:], EXP,
                                bias=pen_sb[:, jc:jc + 1], scale=0.125,
                            )
                        for ih in range(2):
                            nc.tensor.matmul(
                                ps_o[ih][:, 0:512],
                                v_nat[jc][:, ha * 65:(ha + 1) * 65],
                                e_sb[ih][:, 0:512],
                                start=(jc == 0), stop=(jc == n_jc - 1),
                            )
                            nc.tensor.matmul(
                                ps_o[ih][:, 512:1024],
                                v_nat[jc][:, hb * 65:(hb + 1) * 65],
                                e_sb[ih][:, 512:1024],
                                start=(jc == 0), stop=(jc == n_jc - 1),
                            )
                    ots = []
                    for ih in range(2):
                        ot = ot_pool.tile([65, 1024], F32, tag="ot")
                        nc.vector.tensor_copy(ot[:], ps_o[ih][:])
                        ots.append(ot)
                    if pending is not None:
                        epilogue(*pending[0])
                        epilogue(*pending[1])
                    pending = [(p, 0, ots[0]), (p, 1, ots[1])]
                epilogue(*pending[0])
                epilogue(*pending[1])

            if repeat > 0:
                with tc.For_i(0, repeat, 1):
                    body()
            else:
                body()

    nc.compile()
    return nc


def _host_prep(x, mask, w_qkv):
    """Shard + lay out inputs per core. Returns (in_maps, n_j)."""
    x = np.ascontiguousarray(x, dtype=np.float32)
    mask = np.asarray(mask)
    w_qkv = np.ascontiguousarray(w_qkv, dtype=np.float32)

    # kept key rows per batch: j=0 always kept, then mask over rows 1..N-1
    keep = np.concatenate([np.ones((B, 1), dtype=bool), mask.astype(bool)], axis=1)
    counts = keep.sum(axis=1)
    n_j = int(np.ceil(counts.max() / 128.0) * 128)
    n_j = min(n_j, N)

    ident = np.eye(P, dtype=np.float32)
    in_maps = []
    for b in range(B):
        xt = np.ascontiguousarray(x[b].T)               # [D, N]
        idx = np.nonzero(keep[b])[0]
        m = {"xt": xt, "w": w_qkv, "ident": ident}
        pen = np.full(n_j, -10000.0, dtype=np.float32)  # padding rows masked out
        pen[: len(idx)] = 0.0
        m["pen"] = np.ascontiguousarray(pen.reshape(n_j // 128, 128).T)  # [128, n_jc]
        if n_j == N:
            # no gather: full rows, penalty by original position
            penf = np.full(N, -10000.0, dtype=np.float32)
            penf[keep[b]] = 0.0
            m["pen"] = np.ascontiguousarray(penf.reshape(N // 128, 128).T)
        else:
            xkv = np.zeros((D, n_j), dtype=np.float32)
            xkv[:, : len(idx)] = xt[:, idx]
            m["xtkv"] = xkv
        in_maps.append(m)
    return in_maps, n_j


def kernel(x, mask, w_qkv):
    in_maps, n_j = _host_prep(x, mask, w_qkv)
    if n_j not in _CACHE:
        _CACHE[n_j] = build_nc(n_j)
    nc = _CACHE[n_j]
    res = run_bass_kernel_spmd(nc, in_maps, core_ids=list(range(B)))
    out = np.stack([np.asarray(res.results[i]["out"]) for i in range(B)], axis=0)
    return out.astype(np.float32)


if __name__ == "__main__":
    rng = np.random.default_rng(0)
    x = rng.standard_normal((B, N, D), dtype=np.float32)
    mask = rng.integers(0, 2, size=(B, N - 1)).astype(np.int32)
    w = (rng.standard_normal((D, 3 * D), dtype=np.float32) * D ** -0.5).astype(np.float32)
    out = kernel(x=x, mask=mask, w_qkv=w)
    print("out", out.shape, out.dtype, float(np.abs(out).mean()))


# revision 9
# speedup vs baseline: 2.0537x; 1.4314x over previous
"""Trainium2 Bass kernel for batched 16-head attention (B=8, N=1024, D=1024).

Sharding: data-parallel over batch — one batch element per NeuronCore (8 cores).

Per-core pipeline (all matmuls in float32r, fp32 storage):
  1. q,k projected feature-major (qT/kT = W.T @ x.T), v projected seq-major,
     with a ones-column appended per head so the attention-weight row-sums
     (softmax denominators) fall out of the same matmul as out = e.T @ v_aug.
  2. scores computed transposed (scores_T[j, i] = k . q) so the mask penalty
     is a per-partition bias fused into the ScalarE exp together with the
     1/sqrt(d) scale: e = exp(0.125 * scores_T + pen[j]).
  3. masked key rows are dropped entirely (host gathers kept rows; a masked
     row's exp(-10000 + s) is exactly 0.0 in f32, so dropping it is exact).
  4. out_raw_T accumulated per head, PE-transposed back to natural layout,
     normalized by the row-sums, DMA'd out.
"""

import sys

sys.path.insert(0, "/opt/trn_rl_repo")

import numpy as np

import concourse.bass as bass
import concourse.bacc as bacc
import concourse.mybir as mybir
from concourse.tile import TileContext
from concourse.bass_utils import run_bass_kernel_spmd

B = 8
N = 1024          # sequence length (queries)
D = 1024          # model dim
H = 16            # heads
DH = 64           # head dim
NPAIR = H // 2    # head pairs (2 heads share one 128-row feature tile)
P = 128
F32 = mybir.dt.float32
F32R = mybir.dt.float32r
EXP = mybir.ActivationFunctionType.Exp

_CACHE = {}


def build_nc(n_j, repeat=0):
    """Build the per-core Bass graph.

    n_j: padded count of kept key rows (multiple of 128). If n_j == N the
         k/v projections read the full xT input (no separate gathered input).
    repeat: if > 0, wrap the whole compute in a For_i timing loop.

    Structure: projection matmul chains are interleaved into the attention
    jc-loops via a work feeder, so PE fills its exp-latency gaps with proj
    work instead of stalling (PE executes strictly in emission order).
    """
    n_jc = n_j // 128
    share_xt = n_j == N

    nc = bacc.Bacc(None, target_bir_lowering=False)
    xt_ext = nc.declare_dram_parameter("xt", [D, N], F32, isOutput=False)
    if not share_xt:
        xtkv_ext = nc.declare_dram_parameter("xtkv", [D, n_j], F32, isOutput=False)
    w_ext = nc.declare_dram_parameter("w", [D, 3 * D], F32, isOutput=False)
    pen_ext = nc.declare_dram_parameter("pen", [P, n_jc], F32, isOutput=False)
    id_ext = nc.declare_dram_parameter("ident", [P, P], F32, isOutput=False)
    out_ext = nc.declare_dram_parameter("out", [N, D], F32, isOutput=True)

    with TileContext(nc) as tc:
        with (
            tc.tile_pool(name="const", bufs=1) as const_pool,
            tc.tile_pool(name="xt", bufs=1) as xt_pool,
            tc.tile_pool(name="qk", bufs=1) as qk_pool,
            tc.tile_pool(name="vnat", bufs=1) as v_pool,
            tc.tile_pool(name="wq", bufs=3) as w_pool,
            tc.tile_pool(name="wv", bufs=1) as wv_pool,
            tc.tile_pool(name="e", bufs=2) as e_pool,
            tc.tile_pool(name="ot", bufs=2) as ot_pool,
            tc.tile_pool(name="ob", bufs=3) as ob_pool,
            tc.tile_pool(name="pss", bufs=1, space="PSUM") as pss_pool,
            tc.tile_pool(name="pso", bufs=1, space="PSUM") as pso_pool,
            tc.tile_pool(name="psj", bufs=2, space="PSUM") as psj_pool,
        ):
            pen_sb = const_pool.tile([P, n_jc], F32, tag="pen")
            nc.sync.dma_start(out=pen_sb[:], in_=pen_ext[:])
            id_sb = const_pool.tile([P, P], F32, tag="ident")
            nc.sync.dma_start(out=id_sb[:], in_=id_ext[:])

            xt_sb = []
            for dc in range(8):
                t = xt_pool.tile([P, N], F32R, tag=f"xt{dc}")
                for h0 in range(0, N, 512):
                    nc.sync.dma_start(
                        out=t[:, h0:h0 + 512],
                        in_=xt_ext[dc * P:(dc + 1) * P, h0:h0 + 512].bitcast(F32R),
                    )
                xt_sb.append(t)
            if share_xt:
                xtkv_sb = xt_sb
            else:
                xtkv_sb = []
                for dc in range(8):
                    t = xt_pool.tile([P, n_j], F32R, tag=f"xtkv{dc}")
                    nc.sync.dma_start(
                        out=t[:, :], in_=xtkv_ext[dc * P:(dc + 1) * P, :].bitcast(F32R)
                    )
                    xtkv_sb.append(t)

            def body():
                qk_sb = [None] * 16
                v_nat = []
                for jc in range(n_jc):
                    t = v_pool.tile([P, H * 65], F32R, tag=f"v{jc}", name=f"v{jc}")
                    nc.vector.memset(
                        t.rearrange("p (h c) -> p h c", c=65)[:, :, 64:65].bitcast(F32),
                        1.0,
                    )
                    v_nat.append(t)
                wv_sb = {}

                # ---------- projection work units ----------
                def qk_chain(fc):
                    """Yield one closure per PE matmul for projection chain fc."""
                    n_cols = N if fc < 8 else n_j
                    src_ = xt_sb if fc < 8 else xtkv_sb
                    state = {}

                    def first():
                        w_sb = w_pool.tile([P, D], F32R, tag="w", name=f"w{fc}")
                        for dc_ in range(8):
                            nc.sync.dma_start(
                                out=w_sb[:, dc_ * P:(dc_ + 1) * P],
                                in_=w_ext[dc_ * P:(dc_ + 1) * P,
                                          fc * P:(fc + 1) * P].bitcast(F32R),
                            )
                        ps = psj_pool.tile([P, N], F32, tag="proj", name=f"pj{fc}")
                        state["w"] = w_sb
                        state["ps"] = ps

                    halves = [(c0, min(c0 + 512, n_cols)) for c0 in range(0, n_cols, 512)]
                    units = [(hi, dc) for hi in range(len(halves)) for dc in range(8)]

                    def make(i, hi, dc):
                        def emit():
                            if i == 0:
                                first()
                            c0, c1 = halves[hi]
                            nc.tensor.matmul(
                                state["ps"][:, c0:c1],
                                state["w"][:, dc * P:(dc + 1) * P],
                                src_[dc][:, c0:c1],
                                start=(dc == 0), stop=(dc == 7),
                            )
                            if i == len(units) - 1:
                                dst = qk_pool.tile([P, n_cols], F32R,
                                                   tag=f"qk{fc}", name=f"qk{fc}")
                                nc.vector.tensor_copy(dst[:], state["ps"][:, :n_cols])
                                qk_sb[fc] = dst
                        return emit
                    return [make(i, hi, dc) for i, (hi, dc) in enumerate(units)]

                def wv_dma(hv):
                    def emit():
                        tiles = []
                        for dc_ in range(8):
                            t = wv_pool.tile([P, 512], F32R, tag=f"wv{hv}_{dc_}",
                                             name=f"wv{hv}_{dc_}")
                            nc.sync.dma_start(
                                out=t[:],
                                in_=w_ext[dc_ * P:(dc_ + 1) * P,
                                          2048 + hv * 512:2048 + (hv + 1) * 512
                                          ].bitcast(F32R),
                            )
                            tiles.append(t)
                        wv_sb[hv] = tiles
                    return emit

                def v_chain(hv, jc):
                    def make(dc):
                        def emit():
                            if dc == 0:
                                ps = psj_pool.tile([P, 512], F32, tag="proj",
                                                   name=f"pv{hv}_{jc}")
                                v_chain.ps = ps
                            nc.tensor.matmul(
                                v_chain.ps[:],
                                xtkv_sb[dc][:, jc * P:(jc + 1) * P],
                                wv_sb[hv][dc][:],
                                start=(dc == 0), stop=(dc == 7),
                            )
                            if dc == 7:
                                dstv = v_nat[jc].rearrange("p (h c) -> p h c", c=65)
                                nc.vector.tensor_copy(
                                    dstv[:, hv * 8:(hv + 1) * 8, 0:64],
                                    v_chain.ps[:].rearrange("p (h c) -> p h c", c=64),
                                )
                        return emit
                    return [make(dc) for dc in range(8)]

                # ---------- upfront: q0, k0, v(hv=0) ----------
                wv_dma(0)()
                for u in qk_chain(0):
                    u()
                for u in qk_chain(8):
                    u()
                for jc in range(n_jc):
                    for u in v_chain(0, jc):
                        u()

                # ---------- stream: remaining proj work, fed into attention ----
                stream = []
                markers = {}          # pair -> index into stream that must be done
                stream.append(wv_dma(1))
                # v(hv=1) chains must all be emitted before pair NPAIR//2
                # (the first consumer of head slots 8-15): spread over p=1..3.
                vq = list(range(n_jc))
                per_p = -(-len(vq) // 3)
                for p in range(1, NPAIR):
                    stream.extend(qk_chain(p))
                    stream.extend(qk_chain(8 + p))
                    if p <= 3:
                        for jc in vq[(p - 1) * per_p:p * per_p]:
                            stream.extend(v_chain(1, jc))
                    markers[p] = len(stream)
                pos = [0]

                def feed(k):
                    e0 = pos[0]
                    e1 = min(e0 + k, len(stream))
                    for i in range(e0, e1):
                        stream[i]()
                    pos[0] = e1

                def feed_until(idx):
                    while pos[0] < idx:
                        stream[pos[0]]()
                        pos[0] += 1

                # ---------- attention ----------
                def epilogue(p, ih, ot):
                    i0 = ih * 512
                    ps_t = psj_pool.tile([P, 1024], F32, tag="proj",
                                         name=f"pt{p}_{ih}")
                    for ic in range(4):
                        c = ic * P
                        t0 = ic * 256
                        nc.tensor.transpose(
                            ps_t[:, t0:t0 + 65], ot[:, c:c + P], id_sb[0:65, 0:65]
                        )
                        nc.tensor.transpose(
                            ps_t[:, t0 + 128:t0 + 193],
                            ot[:, 512 + c:512 + c + P], id_sb[0:65, 0:65]
                        )
                    for ic in range(4):
                        t0 = ic * 256
                        ob = ob_pool.tile([P, P], F32, tag="ob", name=f"ob{p}_{ih}_{ic}")
                        rc = ob_pool.tile([P, 2], F32, tag="rc", name=f"rc{p}_{ih}_{ic}")
                        nc.vector.reciprocal(rc[:, 0:1], ps_t[:, t0 + 64:t0 + 65])
                        nc.vector.reciprocal(rc[:, 1:2], ps_t[:, t0 + 192:t0 + 193])
                        nc.vector.tensor_scalar_mul(
                            ob[:, 0:64], ps_t[:, t0:t0 + 64], rc[:, 0:1]
                        )
                        nc.vector.tensor_scalar_mul(
                            ob[:, 64:128], ps_t[:, t0 + 128:t0 + 192], rc[:, 1:2]
                        )
                        nc.sync.dma_start(
                            out=out_ext[i0 + ic * P:i0 + (ic + 1) * P,
                                        p * P:(p + 1) * P],
                            in_=ob[:],
                        )

                pending = []
                for p in range(NPAIR):
                    if p in markers:
                        feed_until(markers[p])
                    qT = qk_sb[p]
                    kT = qk_sb[8 + p]
                    ha, hb = 2 * p, 2 * p + 1
                    for ih in range(2):
                        i0 = ih * 512
                        ps_s = pss_pool.tile([P, 1024], F32, tag="s",
                                             name=f"s{p}_{ih}")
                        ps_o = pso_pool.tile([65, 1024], F32, tag="o",
                                             name=f"o{p}_{ih}")
                        for jc in range(n_jc):
                            nc.tensor.matmul(
                                ps_s[:, 0:512],
                                kT[0:64, jc * P:(jc + 1) * P],
                                qT[0:64, i0:i0 + 512],
                                start=True, stop=True,
                            )
                            nc.tensor.matmul(
                                ps_s[:, 512:1024],
                                kT[64:128, jc * P:(jc + 1) * P],
                                qT[64:128, i0:i0 + 512],
                                start=True, stop=True,
                            )
                            e_sb = e_pool.tile([P, 1024], F32R, tag="e",
                                               name=f"e{p}_{ih}_{jc}")
                            nc.scalar.activation(
                                e_sb[:], ps_s[:], EXP,
                                bias=pen_sb[:, jc:jc + 1], scale=0.125,
                            )
                            nc.tensor.matmul(
                                ps_o[:, 0:512],
                                v_nat[jc][:, ha * 65:(ha + 1) * 65],
                                e_sb[:, 0:512],
                                start=(jc == 0), stop=(jc == n_jc - 1),
                            )
                            nc.tensor.matmul(
                                ps_o[:, 512:1024],
                                v_nat[jc][:, hb * 65:(hb + 1) * 65],
                                e_sb[:, 512:1024],
                                start=(jc == 0), stop=(jc == n_jc - 1),
                            )
                            feed(3)
                        ot = ot_pool.tile([65, 1024], F32, tag="ot",
                                          name=f"ot{p}_{ih}")
                        nc.vector.tensor_copy(ot[:], ps_o[:])
                        if pending:
                            epilogue(*pending.pop(0))
                        pending.append((p, ih, ot))
                feed(10 ** 9)
                for args in pending:
                    epilogue(*args)

            if repeat > 0:
                with tc.For_i(0, repeat, 1):
                    body()
            else:
                body()

    nc.compile()
    return nc


def _host_prep(x, mask, w_qkv):
    """Shard + lay out inputs per core. Returns (in_maps, n_j)."""
    x = np.ascontiguousarray(x, dtype=np.float32)
    mask = np.asarray(mask)
    w_qkv = np.ascontiguousarray(w_qkv, dtype=np.float32)

    # kept key rows per batch: j=0 always kept, then mask over rows 1..N-1
    keep = np.concatenate([np.ones((B, 1), dtype=bool), mask.astype(bool)], axis=1)
    counts = keep.sum(axis=1)
    n_j = int(np.ceil(counts.max() / 128.0) * 128)
    n_j = min(n_j, N)

    ident = np.eye(P, dtype=np.float32)
    in_maps = []
    for b in range(B):
        xt = np.ascontiguousarray(x[b].T)               # [D, N]
        idx = np.nonzero(keep[b])[0]
        m = {"xt": xt, "w": w_qkv, "ident": ident}
        pen = np.full(n_j, -10000.0, dtype=np.float32)  # padding rows masked out
        pen[: len(idx)] = 0.0
        m["pen"] = np.ascontiguousarray(pen.reshape(n_j // 128, 128).T)  # [128, n_jc]
        if n_j == N:
            # no gather: full rows, penalty by original position
            penf = np.full(N, -10000.0, dtype=np.float32)
            penf[keep[b]] = 0.0
            m["pen"] = np.ascontiguousarray(penf.reshape(N // 128, 128).T)
        else:
            xkv = np.zeros((D, n_j), dtype=np.float32)
            xkv[:, : len(idx)] = xt[:, idx]
            m["xtkv"] = xkv
        in_maps.append(m)
    return in_maps, n_j


def kernel(x, mask, w_qkv):
    in_maps, n_j = _host_prep(x, mask, w_qkv)
    if n_j not in _CACHE:
        _CACHE[n_j] = build_nc(n_j)
    nc = _CACHE[n_j]
    res = run_bass_kernel_spmd(nc, in_maps, core_ids=list(range(B)))
    out = np.stack([np.asarray(res.results[i]["out"]) for i in range(B)], axis=0)
    return out.astype(np.float32)


if __name__ == "__main__":
    rng = np.random.default_rng(0)
    x = rng.standard_normal((B, N, D), dtype=np.float32)
    mask = rng.integers(0, 2, size=(B, N - 1)).astype(np.int32)
    w = (rng.standard_normal((D, 3 * D), dtype=np.float32) * D ** -0.5).astype(np.float32)
    out = kernel(x=x, mask=mask, w_qkv=w)
    print("out", out.shape, out.dtype, float(np.abs(out).mean()))
